# revision 41
# baseline (speedup 1.0000x reference)
"""CWT (continuous wavelet transform, pywt 'morl', 5 scales) as a Bass/Tile
kernel for 8 Trainium2 NeuronCores.

Math: for each scale s with integrated-wavelet filter k (length L), the
reference computes  trim(diff(full_corr(x, k))) * (-sqrt(s)) along T.  That
whole pipeline is a single correlation with the fixed kernel
    G[j] = sqrt(s) * (k[j] - k[j-1]),  j = 0..L  (k[-1] = k[L] = 0)
applied with offset  off = floor((L-2)/2) - (L-1):
    y[t] = sum_j x[t + off + j] * G[j]   (x zero-padded outside [0,T))
i.e. y = A_s @ x with the Toeplitz band matrix A_s[t, u] = G[u - t - off].

Kernel strategy v3 (SPMD over 8 cores): pure B-sharding.  Core c owns the
128 batch*channel columns [128c, 128c+128); every core computes all 2048
t_out rows for its columns, as banded Toeplitz matmuls in bf16:
    psum[b, c0:c1] += X_chunk[q].T @ Wsc[:, w0(s,I,q)+c0 : w0+c1]
with per-window nonzero-band column trimming (streams ~90k PE columns).

v3 schedule is BLOCK-MAJOR: t_out blocks are processed [3, 2, 1, 0]; inside
a block the t_in chunks q sweep ascending, and at each (block, q) all scales
with an active band window are emitted back-to-back so they share the
stationary x-chunk (LDWEIGHTS pressure drops and the PE stream stays dense
even through the small-N scale-1 windows).  Each chain's first (lowest-q)
matmul is widened to the chain's column union and carries start; its
highest-q matmul carries stop.  Processing block 0 last staggers the chain
endpoints (s0 stops 11 matmuls before the end, s1 10, s2 7, s3 1), so all
drains except the very last s4 chain overlap compute; that final copy+store
is split in halves across both DMA rings to minimize the tail.

DMA: x (0.5MB) and W (2.3MB band columns only) in bf16 on the two HWDGE
rings (sync=x+stores, scalar=W+late stores), consumption-ordered, with the
leading x/W pieces cut small so the first matmul issues right after engine
boot.  Outputs are staged psum->SBUF (vector cast to bf16) per chain; the
two first-processed blocks (3,2) of each scale are stored as one merged
1024-col DMA, blocks 1 and 0 per-block, spread across both rings.
"""
import sys
import os

sys.path.insert(0, "/opt/trn_rl_repo")

import numpy as np

# ----------------------------------------------------------------- constants
WIDTHS = [1, 27, 76, 167, 336]
T = 2048
B = 1024  # 16 batch * 64 channels
N_CORES = 8
BPC = B // N_CORES  # 128 batch*channel columns per core
NQ = T // 128  # 16 t_in chunks
NI = T // 512  # 4 t_out blocks per core (all computed by every core)
NSC = len(WIDTHS)

BLOCK_ORDER = [3, 2, 1, 0]  # block 0 last -> staggered chain endings
LAST_CHAIN = (4, 0)  # final chain: s4 of block 0 (split copy/store tail)

LAST_EXEC_NS = None  # set when CWT_TRACE=1


def _filters():
    """pywt 'morl' integrated wavelet, resampled per scale (matches reference)."""
    precision = 10
    n = 2**precision
    lb, ub = -8.0, 8.0
    t = np.linspace(lb, ub, n)
    psi = np.exp(-(t**2) / 2.0) * np.cos(5.0 * t)
    step = t[1] - t[0]
    int_psi = np.cumsum(psi) * step
    filts = []
    for scale in WIDTHS:
        j = (np.arange(scale * (ub - lb) + 1) / (scale * step)).astype(np.int64)
        j = j[j < n]
        filts.append(int_psi[j].astype(np.float32))
    return filts


def _g_kernels():
    """Effective correlation kernels G_s (len L+1) and offsets off_s."""
    gs = []
    for s, k in zip(WIDTHS, _filters()):
        k64 = k.astype(np.float64)
        L = len(k64)
        G = (np.sqrt(s) * np.diff(np.concatenate([[0.0], k64, [0.0]]))).astype(
            np.float32
        )
        off = int(np.floor((L - 2) / 2.0)) - (L - 1)
        gs.append((G, off))
    return gs


def _plan():
    """Per-scale Toeplitz window geometry + per-block chunk ranges.

    w0(s, I, q) = C_s + off_s - (128q - 512I) is the first W column of the
    512-wide rhs slice for chunk q of t_out block I.
    """
    plans = []
    for G, off in _g_kernels():
        L1 = len(G)
        qr = []
        vs = []
        for I in range(NI):
            lo = max(0, (512 * I + off) // 128)
            hi = min(NQ - 1, (512 * I + 511 + off + L1 - 1) // 128)
            qr.append((lo, hi))
            vs += [128 * q - 512 * I for q in range(lo, hi + 1)]
        C = max(vs) - off
        W = max(vs) - min(vs) + 512
        plans.append({"off": off, "L1": L1, "qr": qr, "C": C, "W": W, "G": G})
    return plans


def _toeplitz(G, C, W):
    p = np.arange(128)[:, None]
    w = np.arange(W)[None, :]
    idx = p - w + C
    valid = (idx >= 0) & (idx < len(G))
    return np.where(valid, G[np.clip(idx, 0, len(G) - 1)], np.float32(0.0)).astype(
        np.float32
    )


def _chain_windows(p, I):
    """q-ascending banded column windows for one (scale, block) chain.

    Returns [(q, c0, c1, start, stop)]: matmul psum cols [c0, c1), skipping
    the all-zero columns of the Toeplitz band slice.  The first (lowest-q)
    window carries start; start_tensor_calc resets the ENTIRE psum bank, so
    later windows may touch columns outside the start window's span (every
    chain's window union is exactly [0,512), asserted here, so the full-bank
    copy after stop only reads reset-or-written columns).  The last
    (highest-q) window carries stop.
    """
    C, L1, off = p["C"], p["L1"], p["off"]
    nz_lo, nz_hi = C - L1 + 1, C + 128  # nonzero W cols [lo, hi)
    lo, hi = p["qr"][I]
    spans = {}
    for q in range(lo, hi + 1):
        w0 = C + off - (128 * q - 512 * I)
        a = max(0, nz_lo - w0)
        b = min(512, nz_hi - w0)
        if b <= a:
            continue
        spans[q] = (a, b)
    qs = sorted(spans)
    assert (min(a for a, _ in spans.values()), max(b for _, b in spans.values())) == (
        0,
        512,
    ), (I, spans)
    return [
        (q, spans[q][0], spans[q][1], q == qs[0], q == qs[-1]) for q in qs
    ]


def _schedule(plans):
    """Emission-ordered list of (chain, q, w0, c0, c1, start, stop,
    chain_done) where chain = (s, I).

    Block-major: blocks in BLOCK_ORDER; q ascending inside a block; scales
    ascending inside a q-group (so the stationary x-chunk is shared by
    consecutive matmuls, and s4 - the only chain alive at the final q of
    the last block - is emitted last).
    """
    sched = []
    for I in BLOCK_ORDER:
        wins = {s: {w[0]: w for w in _chain_windows(plans[s], I)} for s in range(NSC)}
        for q in range(NQ):
            for s in range(NSC):
                w = wins[s].get(q)
                if w is None:
                    continue
                _, c0, c1, st, sp = w
                w0 = plans[s]["C"] + plans[s]["off"] - (128 * q - 512 * I)
                sched.append(((s, I), q, w0, c0, c1, st, sp, sp))
    return sched


_CONST_CACHE = None


def _consts():
    global _CONST_CACHE
    if _CONST_CACHE is None:
        import ml_dtypes

        plans = _plan()
        wbuf = np.concatenate(
            [_toeplitz(p["G"], p["C"], p["W"]) for p in plans], axis=1
        ).astype(ml_dtypes.bfloat16)
        _CONST_CACHE = (plans, np.ascontiguousarray(wbuf))
    return _CONST_CACHE


# ----------------------------------------------------------------- program
_NC_CACHE = None


# DMA model constants (measured): issue+descriptor cost per piece, ring
# bandwidth, PE column rate, and the DMA completion-post queue: semaphore
# posts are processed serially at ~_POST_GAP_US apiece when completions
# bunch, so data of the k-th completing piece is not usable before the
# k-th post slot no matter how small the piece is.
_DMA_ISSUE_US = 0.62
_DMA_BPUS = 0.185e6  # bytes per us per ring
_POST_GAP_US = 0.78
_SEM2ENG_US = 0.40
_PE_COL_US = 0.00046
_PE_T0_US = 9.3  # first possible matmul issue after boot
_SAFETY_US = 0.15

# ring per load stream: x and the W of s3/s1 on sync, W of s4/s2/s0 on
# scalar (nearly all of W is consumed during the first block's sweep, so a
# single ring cannot feed it)
_W_RING = {0: "scalar", 1: "sync", 2: "scalar", 3: "sync", 4: "scalar"}


def _load_pieces(plans, sched):
    """All input DMA pieces (x chunks + W columns) with ring assignment,
    in global first-use order: [(ring, kind, lo, hi)], kind 'x' (chunk
    units) or 'w' (concat W cols).

    This reproduces the measured-best layout: x on the sync ring in four
    pieces; W on the scalar ring, per scale, with the first two scales'
    leading piece trimmed to their first two windows (~900/750 cols) and
    the rest in ~1024-col cuts.  The piece count is kept low and the
    per-piece first uses are spaced, which matches two hardware limits:
    8 HW DMA semaphores recycle by emission index, and completion posts
    drain through a serial ~0.8us queue.
    """
    bases = []
    b = 0
    for p in plans:
        bases.append(b)
        b += p["W"]
    pieces = []  # (first_use_idx, tiebreak, ring, kind, lo, hi)
    xq_first = {}
    for i, (c, q, w0, c0, c1, st, sp, cd) in enumerate(sched):
        xq_first.setdefault(q, i)
    for g0, g1 in ((0, 2), (2, 5), (5, 9), (9, NQ)):
        first = min(xq_first[q] for q in range(g0, g1))
        pieces.append((first, 0, "sync", "x", g0, g1))
    lead = []
    for c, q, w0, c0, c1, st, sp, cd in sched:
        if c[0] not in lead:
            lead.append(c[0])
    for s, p in enumerate(plans):
        rd = [
            (i, w0 + c0, w0 + c1)
            for i, (c, q, w0, c0, c1, st, sp, cd) in enumerate(sched)
            if c[0] == s
        ]
        rlo = min(a for _, a, b in rd)
        rhi = max(b for _, a, b in rd)
        cuts = set()
        if s in lead[:2]:
            first2 = sorted(rd)[:2]
            flo = min(a for _, a, b in first2)
            cuts.add(((flo - rlo) & ~127) + rlo)
        npieces = max(1, round((rhi - rlo) / 1024))
        step = (((rhi - rlo) // npieces) + 127) & ~127
        cuts |= set(range(rlo + step, rhi, step))
        allcuts = [rlo] + sorted(c for c in cuts if rlo < c < rhi) + [rhi]
        for lo, hi in zip(allcuts[:-1], allcuts[1:]):
            first = min(i for i, a, b in rd if a < hi and b > lo)
            pieces.append((first, 1, "scalar", "w", bases[s] + lo, bases[s] + hi))
    pieces.sort()
    return bases, [(r, k, lo, hi) for _, _, r, k, lo, hi in pieces]


def _build_program():
    import concourse.bass as bass
    import concourse.bacc as bacc
    import concourse.mybir as mybir
    import concourse.tile as tile

    plans, _ = _consts()
    sched = _schedule(plans)
    wtot = sum(p["W"] for p in plans)
    bases, lpieces = _load_pieces(plans, sched)

    # psum bank per chain: 5 banks per block, rotating through the 8 banks
    # across the 4 processed blocks so every reuse has >= 1 block of slack
    bank = {}
    nb = 0
    for I in BLOCK_ORDER:
        for s in range(NSC):
            bank[(s, I)] = nb % 8
            nb += 1

    # store plan (few, large DMAs — see the 8-semaphore note above): the
    # small scales stop early within block 0 (s0 at sweep q4, s1 q5, s2
    # q8), so each gets ONE whole-scale store there; s3/s4 store merged
    # blocks 3+2 mid-kernel, then block 1, then block 0 in the tail.
    # Rings roughly balanced; block-0 stores keep scalar free for the tail.
    STORE_ENG = {
        (0, "all"): "scalar",
        (1, "all"): "sync",
        (2, "all"): "scalar",
        (3, "m32"): "scalar",
        (3, 1): "sync",
        (3, 0): "sync",
        (4, "m32"): "scalar",
        (4, 1): "sync",
    }

    nc = bacc.Bacc(None, target_bir_lowering=False, debug=False)

    x_d = nc.declare_dram_parameter("x", [128, NQ * BPC], mybir.dt.bfloat16, isOutput=False)
    w_d = nc.declare_dram_parameter("w", [128, wtot], mybir.dt.bfloat16, isOutput=False)
    # outputs leave as bf16 (host upcasts): halves the store bytes; adds
    # ~0.1% rms quantization vs the 2e-2 budget
    out_d = nc.declare_dram_parameter(
        "out", [NSC, 128, T], mybir.dt.bfloat16, isOutput=True
    )

    with tile.TileContext(nc) as tc:
        with (
            tc.tile_pool(name="sb", bufs=1) as sb,
            tc.tile_pool(name="pp", bufs=1, space=bass.MemorySpace.PSUM) as pp,
        ):
            # nearly all of W is consumed already during the first block's
            # sweep, so one ring cannot feed it: split the load streams by
            # scale across both rings (sync: x + W of s3/s1, scalar: W of
            # s4/s2/s0), each in first-use order with small leads so the
            # opening matmuls are not DMA-gated
            xsb = sb.tile([128, NQ * BPC], mybir.dt.bfloat16, tag="xsb", name="xsb")
            wsb = sb.tile([128, wtot], mybir.dt.bfloat16, tag="wsb", name="wsb")

            # PE p-state warmup: the tensor engine runs at ~1.2GHz for its
            # first ~3.4us of activity.  The real stream cannot start until
            # the first DMA pieces land (~2us after engine boot), so burn
            # that dead window with dummy matmuls on a memset scratch tile
            # — the ramp window is then mostly spent before real work.
            scr = sb.tile([128, 512], mybir.dt.bfloat16, tag="scr", name="scr")
            pwarm = pp.tile([128, 512], mybir.dt.float32, tag="ps7", name="ps_warm")
            nc.gpsimd.memset(scr[:], 0.0)
            for _ in range(4):
                nc.tensor.matmul(
                    pwarm[:], scr[:, 0:128], scr[:], start=True, stop=True
                )

            for ring, kind, lo, hi in lpieces:
                eng = getattr(nc, ring)
                if kind == "x":
                    eng.dma_start(
                        xsb[:, lo * BPC : hi * BPC], x_d[:, lo * BPC : hi * BPC]
                    )
                else:
                    eng.dma_start(wsb[:, lo:hi], w_d[:, lo:hi])

            stgs = [
                sb.tile([128, T], mybir.dt.bfloat16, tag=f"stg{s}", name=f"stg{s}")
                for s in range(NSC)
            ]

            psums = {}
            for (s, I), bk in bank.items():
                psums[(s, I)] = pp.tile(
                    [128, 512],
                    mybir.dt.float32,
                    tag=f"ps{bk}",
                    name=f"ps_{s}_{I}",
                )

            for c, q, w0, c0, c1, start, stop, chain_done in sched:
                s, I = c
                nc.tensor.matmul(
                    psums[c][:, c0:c1],
                    xsb[:, q * BPC : (q + 1) * BPC],
                    wsb[:, bases[s] + w0 + c0 : bases[s] + w0 + c1],
                    start=start,
                    stop=stop,
                )
                if not chain_done:
                    continue
                stg = stgs[s]
                t0c = 512 * I
                if c == LAST_CHAIN:
                    # final chain: one DVE cast (a single 512-col cast beats
                    # two serial 256-col halves), stores split on both rings
                    nc.vector.tensor_copy(
                        stg[:, t0c : t0c + 512], psums[c][:, 0:512]
                    )
                    nc.sync.dma_start(
                        out_d[s][:, t0c : t0c + 256], stg[:, t0c : t0c + 256]
                    )
                    nc.scalar.dma_start(
                        out_d[s][:, t0c + 256 : t0c + 512],
                        stg[:, t0c + 256 : t0c + 512],
                    )
                    continue
                nc.vector.tensor_copy(
                    stg[:, t0c : t0c + 512], psums[c][:, 0:512]
                )
                if s <= 2:
                    if I == BLOCK_ORDER[3]:  # last block: whole scale ready
                        eng = getattr(nc, STORE_ENG[(s, "all")])
                        eng.dma_start(out_d[s], stg[:])
                    continue
                if I == BLOCK_ORDER[0]:
                    continue  # held in stg; stored merged with next block
                if I == BLOCK_ORDER[1]:
                    # merged store of the two first-processed blocks
                    a = 512 * min(I, BLOCK_ORDER[0])
                    b = 512 * max(I, BLOCK_ORDER[0]) + 512
                    eng = getattr(nc, STORE_ENG[(s, "m32")])
                    eng.dma_start(out_d[s][:, a:b], stg[:, a:b])
                else:
                    eng = getattr(nc, STORE_ENG[(s, I)])
                    eng.dma_start(
                        out_d[s][:, t0c : t0c + 512], stg[:, t0c : t0c + 512]
                    )

    nc.compile()
    return nc


def _program():
    global _NC_CACHE
    if _NC_CACHE is None:
        _NC_CACHE = _build_program()
    return _NC_CACHE


# ----------------------------------------------------------------- entry
def kernel(x: np.ndarray) -> np.ndarray:
    """x: [16, 2048, 64] float32 -> [16, 2048, 64, 5] float32"""
    global LAST_EXEC_NS
    import ml_dtypes
    from concourse.bass_utils import run_bass_kernel_spmd

    x = np.asarray(x)
    n, t, c = x.shape
    assert (t, n * c) == (T, B), (x.shape,)

    X = x.transpose(1, 0, 2).reshape(T, B).astype(np.float32)
    _, wbuf = _consts()
    in_maps = []
    for core in range(N_CORES):
        xc = X[:, core * BPC : (core + 1) * BPC]  # [2048, 128]
        xc = (
            xc.reshape(NQ, 128, BPC)
            .transpose(1, 0, 2)
            .reshape(128, NQ * BPC)
            .astype(ml_dtypes.bfloat16)
        )
        in_maps.append({"x": np.ascontiguousarray(xc), "w": wbuf})

    nc = _program()
    trace = bool(int(os.environ.get("CWT_TRACE", "0")))
    res = run_bass_kernel_spmd(nc, in_maps, list(range(N_CORES)), trace=trace)
    if trace:
        LAST_EXEC_NS = res.exec_time_ns
        globals()["LAST_RESULTS"] = res

    # per-core out: [5, 128, 2048] bf16 (b-local, t) -> Y [5, T, B] fp32
    Y = np.empty((NSC, T, B), np.float32)
    for core in range(N_CORES):
        o = np.asarray(res.results[core]["out"]).astype(np.float32)
        Y[:, :, core * BPC : (core + 1) * BPC] = o.transpose(0, 2, 1)
    return np.ascontiguousarray(
        Y.reshape(NSC, T, n, c).transpose(2, 1, 3, 0).astype(np.float32)
    )


# revision 43
# speedup vs baseline: 1.0059x; 1.0059x over previous
"""CWT (continuous wavelet transform, pywt 'morl', 5 scales) as a Bass/Tile
kernel for 8 Trainium2 NeuronCores.

Math: for each scale s with integrated-wavelet filter k (length L), the
reference computes  trim(diff(full_corr(x, k))) * (-sqrt(s)) along T.  That
whole pipeline is a single correlation with the fixed kernel
    G[j] = sqrt(s) * (k[j] - k[j-1]),  j = 0..L  (k[-1] = k[L] = 0)
applied with offset  off = floor((L-2)/2) - (L-1):
    y[t] = sum_j x[t + off + j] * G[j]   (x zero-padded outside [0,T))
i.e. y = A_s @ x with the Toeplitz band matrix A_s[t, u] = G[u - t - off].

Kernel strategy v3 (SPMD over 8 cores): pure B-sharding.  Core c owns the
128 batch*channel columns [128c, 128c+128); every core computes all 2048
t_out rows for its columns, as banded Toeplitz matmuls in bf16:
    psum[b, c0:c1] += X_chunk[q].T @ Wsc[:, w0(s,I,q)+c0 : w0+c1]
with per-window nonzero-band column trimming (streams ~90k PE columns).

v3 schedule is BLOCK-MAJOR: t_out blocks are processed [3, 2, 1, 0]; inside
a block the t_in chunks q sweep ascending, and at each (block, q) all scales
with an active band window are emitted back-to-back so they share the
stationary x-chunk (the PE pulls each LDWEIGHTS ahead into the background
weight buffer, so the stream stays dense even through small-N windows).
Chains carry start on their first (lowest-q) window with its NATURAL
(untrimmed-to-union) span — start_tensor_calc resets the entire psum bank,
verified on hardware — which keeps the stream at the formulation's column
floor (~90.9k vs 95.9k with union-widened starts).  Processing block 0 last
staggers the chain endpoints (s0 stops 11 matmuls before the end, s1 10,
s2 7, s3 1), so all drains except the very last s4 chain overlap compute;
that final chain gets one DVE cast and half-stores on both rings.

DMA (measured constraints: ~0.6us issue per piece, ~0.185MB/us per ring,
completion-semaphore posts drain through a serial ~0.8us queue, and only 8
HW DMA semaphores exist — recycling makes DMA k wait on DMA k-8's
completion): x (0.5MB) rides the sync ring in four pieces; W (2.25MB, only
band columns actually read) rides the scalar ring with the first two
scales' lead pieces trimmed to their first windows and ~1024-col cuts
after, all in global first-use order.  Outputs are staged psum->SBUF
(vector cast to bf16): s0/s1/s2 store once as whole scales when their
(early-stopping) block-0 chains finish; s3/s4 store blocks 3+2 merged,
then block 1, then block 0 in the tail.
"""
import sys
import os

sys.path.insert(0, "/opt/trn_rl_repo")

import numpy as np

# ----------------------------------------------------------------- constants
WIDTHS = [1, 27, 76, 167, 336]
T = 2048
B = 1024  # 16 batch * 64 channels
N_CORES = 8
BPC = B // N_CORES  # 128 batch*channel columns per core
NQ = T // 128  # 16 t_in chunks
NI = T // 512  # 4 t_out blocks per core (all computed by every core)
NSC = len(WIDTHS)

BLOCK_ORDER = [3, 2, 1, 0]  # block 0 last -> staggered chain endings
LAST_CHAIN = (4, 0)  # final chain: s4 of block 0 (split copy/store tail)

LAST_EXEC_NS = None  # set when CWT_TRACE=1


def _filters():
    """pywt 'morl' integrated wavelet, resampled per scale (matches reference)."""
    precision = 10
    n = 2**precision
    lb, ub = -8.0, 8.0
    t = np.linspace(lb, ub, n)
    psi = np.exp(-(t**2) / 2.0) * np.cos(5.0 * t)
    step = t[1] - t[0]
    int_psi = np.cumsum(psi) * step
    filts = []
    for scale in WIDTHS:
        j = (np.arange(scale * (ub - lb) + 1) / (scale * step)).astype(np.int64)
        j = j[j < n]
        filts.append(int_psi[j].astype(np.float32))
    return filts


def _g_kernels():
    """Effective correlation kernels G_s (len L+1) and offsets off_s."""
    gs = []
    for s, k in zip(WIDTHS, _filters()):
        k64 = k.astype(np.float64)
        L = len(k64)
        G = (np.sqrt(s) * np.diff(np.concatenate([[0.0], k64, [0.0]]))).astype(
            np.float32
        )
        off = int(np.floor((L - 2) / 2.0)) - (L - 1)
        gs.append((G, off))
    return gs


def _plan():
    """Per-scale Toeplitz window geometry + per-block chunk ranges.

    w0(s, I, q) = C_s + off_s - (128q - 512I) is the first W column of the
    512-wide rhs slice for chunk q of t_out block I.
    """
    plans = []
    for G, off in _g_kernels():
        L1 = len(G)
        qr = []
        vs = []
        for I in range(NI):
            lo = max(0, (512 * I + off) // 128)
            hi = min(NQ - 1, (512 * I + 511 + off + L1 - 1) // 128)
            qr.append((lo, hi))
            vs += [128 * q - 512 * I for q in range(lo, hi + 1)]
        C = max(vs) - off
        W = max(vs) - min(vs) + 512
        plans.append({"off": off, "L1": L1, "qr": qr, "C": C, "W": W, "G": G})
    return plans


def _toeplitz(G, C, W):
    p = np.arange(128)[:, None]
    w = np.arange(W)[None, :]
    idx = p - w + C
    valid = (idx >= 0) & (idx < len(G))
    return np.where(valid, G[np.clip(idx, 0, len(G) - 1)], np.float32(0.0)).astype(
        np.float32
    )


def _chain_windows(p, I):
    """q-ascending banded column windows for one (scale, block) chain.

    Returns [(q, c0, c1, start, stop)]: matmul psum cols [c0, c1), skipping
    the all-zero columns of the Toeplitz band slice.  The first (lowest-q)
    window carries start; start_tensor_calc resets the ENTIRE psum bank, so
    later windows may touch columns outside the start window's span (every
    chain's window union is exactly [0,512), asserted here, so the full-bank
    copy after stop only reads reset-or-written columns).  The last
    (highest-q) window carries stop.
    """
    C, L1, off = p["C"], p["L1"], p["off"]
    nz_lo, nz_hi = C - L1 + 1, C + 128  # nonzero W cols [lo, hi)
    lo, hi = p["qr"][I]
    spans = {}
    for q in range(lo, hi + 1):
        w0 = C + off - (128 * q - 512 * I)
        a = max(0, nz_lo - w0)
        b = min(512, nz_hi - w0)
        if b <= a:
            continue
        spans[q] = (a, b)
    qs = sorted(spans)
    assert (min(a for a, _ in spans.values()), max(b for _, b in spans.values())) == (
        0,
        512,
    ), (I, spans)
    return [
        (q, spans[q][0], spans[q][1], q == qs[0], q == qs[-1]) for q in qs
    ]


def _schedule(plans):
    """Emission-ordered list of (chain, q, w0, c0, c1, start, stop,
    chain_done) where chain = (s, I).

    Block-major: blocks in BLOCK_ORDER; q ascending inside a block; scales
    ascending inside a q-group (so the stationary x-chunk is shared by
    consecutive matmuls, and s4 - the only chain alive at the final q of
    the last block - is emitted last).
    """
    sched = []
    for I in BLOCK_ORDER:
        wins = {s: {w[0]: w for w in _chain_windows(plans[s], I)} for s in range(NSC)}
        for q in range(NQ):
            for s in range(NSC):
                w = wins[s].get(q)
                if w is None:
                    continue
                _, c0, c1, st, sp = w
                w0 = plans[s]["C"] + plans[s]["off"] - (128 * q - 512 * I)
                sched.append(((s, I), q, w0, c0, c1, st, sp, sp))
    return sched


_CONST_CACHE = None


def _consts():
    global _CONST_CACHE
    if _CONST_CACHE is None:
        import ml_dtypes

        plans = _plan()
        wbuf = np.concatenate(
            [_toeplitz(p["G"], p["C"], p["W"]) for p in plans], axis=1
        ).astype(ml_dtypes.bfloat16)
        _CONST_CACHE = (plans, np.ascontiguousarray(wbuf))
    return _CONST_CACHE


# ----------------------------------------------------------------- program
_NC_CACHE = None


# DMA model constants (measured): issue+descriptor cost per piece, ring
# bandwidth, PE column rate, and the DMA completion-post queue: semaphore
# posts are processed serially at ~_POST_GAP_US apiece when completions
# bunch, so data of the k-th completing piece is not usable before the
# k-th post slot no matter how small the piece is.
_DMA_ISSUE_US = 0.62
_DMA_BPUS = 0.185e6  # bytes per us per ring
_POST_GAP_US = 0.78
_SEM2ENG_US = 0.40
_PE_COL_US = 0.00046
_PE_T0_US = 9.3  # first possible matmul issue after boot
_SAFETY_US = 0.15

# ring per load stream: x and the W of s3/s1 on sync, W of s4/s2/s0 on
# scalar (nearly all of W is consumed during the first block's sweep, so a
# single ring cannot feed it)
_W_RING = {0: "scalar", 1: "sync", 2: "scalar", 3: "sync", 4: "scalar"}


def _load_pieces(plans, sched):
    """All input DMA pieces (x chunks + W columns) with ring assignment,
    in global first-use order: [(ring, kind, lo, hi)], kind 'x' (chunk
    units) or 'w' (concat W cols).

    This reproduces the measured-best layout: x on the sync ring in four
    pieces; W on the scalar ring, per scale, with the first two scales'
    leading piece trimmed to their first two windows (~900/750 cols) and
    the rest in ~1024-col cuts.  The piece count is kept low and the
    per-piece first uses are spaced, which matches two hardware limits:
    8 HW DMA semaphores recycle by emission index, and completion posts
    drain through a serial ~0.8us queue.
    """
    bases = []
    b = 0
    for p in plans:
        bases.append(b)
        b += p["W"]
    pieces = []  # (first_use_idx, tiebreak, ring, kind, lo, hi)
    xq_first = {}
    for i, (c, q, w0, c0, c1, st, sp, cd) in enumerate(sched):
        xq_first.setdefault(q, i)
    for g0, g1 in ((0, 2), (2, 5), (5, 9), (9, NQ)):
        first = min(xq_first[q] for q in range(g0, g1))
        pieces.append((first, 0, "sync", "x", g0, g1))
    lead = []
    for c, q, w0, c0, c1, st, sp, cd in sched:
        if c[0] not in lead:
            lead.append(c[0])
    for s, p in enumerate(plans):
        rd = [
            (i, w0 + c0, w0 + c1)
            for i, (c, q, w0, c0, c1, st, sp, cd) in enumerate(sched)
            if c[0] == s
        ]
        rlo = min(a for _, a, b in rd)
        rhi = max(b for _, a, b in rd)
        cuts = set()
        if s in lead[:2]:
            first2 = sorted(rd)[:2]
            flo = min(a for _, a, b in first2)
            cuts.add(((flo - rlo) & ~127) + rlo)
        npieces = max(1, round((rhi - rlo) / 1024))
        step = (((rhi - rlo) // npieces) + 127) & ~127
        cuts |= set(range(rlo + step, rhi, step))
        allcuts = [rlo] + sorted(c for c in cuts if rlo < c < rhi) + [rhi]
        for lo, hi in zip(allcuts[:-1], allcuts[1:]):
            first = min(i for i, a, b in rd if a < hi and b > lo)
            pieces.append((first, 1, "scalar", "w", bases[s] + lo, bases[s] + hi))
    pieces.sort()
    return bases, [(r, k, lo, hi) for _, _, r, k, lo, hi in pieces]


def _build_program():
    import concourse.bass as bass
    import concourse.bacc as bacc
    import concourse.mybir as mybir
    import concourse.tile as tile

    plans, _ = _consts()
    sched = _schedule(plans)
    wtot = sum(p["W"] for p in plans)
    bases, lpieces = _load_pieces(plans, sched)

    # psum bank per chain: 5 banks per block, rotating through the 8 banks
    # across the 4 processed blocks so every reuse has >= 1 block of slack
    bank = {}
    nb = 0
    for I in BLOCK_ORDER:
        for s in range(NSC):
            bank[(s, I)] = nb % 8
            nb += 1

    # store plan (few, large DMAs — see the 8-semaphore note above): the
    # small scales stop early within block 0 (s0 at sweep q4, s1 q5, s2
    # q8), so each gets ONE whole-scale store there; s3/s4 store merged
    # blocks 3+2 mid-kernel, then block 1, then block 0 in the tail.
    # Rings roughly balanced; block-0 stores keep scalar free for the tail.
    STORE_ENG = {
        (0, "all"): "scalar",
        (1, "all"): "sync",
        (2, "all"): "scalar",
        (3, "m32"): "scalar",
        (3, 1): "sync",
        (3, 0): "sync",
        (4, "m32"): "scalar",
        (4, 1): "sync",
    }

    nc = bacc.Bacc(None, target_bir_lowering=False, debug=False)

    x_d = nc.declare_dram_parameter("x", [128, NQ * BPC], mybir.dt.bfloat16, isOutput=False)
    w_d = nc.declare_dram_parameter("w", [128, wtot], mybir.dt.bfloat16, isOutput=False)
    # outputs leave as bf16 (host upcasts): halves the store bytes; adds
    # ~0.1% rms quantization vs the 2e-2 budget
    out_d = nc.declare_dram_parameter(
        "out", [NSC, 128, T], mybir.dt.bfloat16, isOutput=True
    )

    with tile.TileContext(nc) as tc:
        with (
            tc.tile_pool(name="sb", bufs=1) as sb,
            tc.tile_pool(name="pp", bufs=1, space=bass.MemorySpace.PSUM) as pp,
        ):
            # nearly all of W is consumed already during the first block's
            # sweep, so one ring cannot feed it: split the load streams by
            # scale across both rings (sync: x + W of s3/s1, scalar: W of
            # s4/s2/s0), each in first-use order with small leads so the
            # opening matmuls are not DMA-gated
            xsb = sb.tile([128, NQ * BPC], mybir.dt.bfloat16, tag="xsb", name="xsb")
            wsb = sb.tile([128, wtot], mybir.dt.bfloat16, tag="wsb", name="wsb")
            for ring, kind, lo, hi in lpieces:
                eng = getattr(nc, ring)
                if kind == "x":
                    eng.dma_start(
                        xsb[:, lo * BPC : hi * BPC], x_d[:, lo * BPC : hi * BPC]
                    )
                else:
                    eng.dma_start(wsb[:, lo:hi], w_d[:, lo:hi])

            stgs = [
                sb.tile([128, T], mybir.dt.bfloat16, tag=f"stg{s}", name=f"stg{s}")
                for s in range(NSC)
            ]

            psums = {}
            for (s, I), bk in bank.items():
                psums[(s, I)] = pp.tile(
                    [128, 512],
                    mybir.dt.float32,
                    tag=f"ps{bk}",
                    name=f"ps_{s}_{I}",
                )

            for c, q, w0, c0, c1, start, stop, chain_done in sched:
                s, I = c
                nc.tensor.matmul(
                    psums[c][:, c0:c1],
                    xsb[:, q * BPC : (q + 1) * BPC],
                    wsb[:, bases[s] + w0 + c0 : bases[s] + w0 + c1],
                    start=start,
                    stop=stop,
                )
                if not chain_done:
                    continue
                stg = stgs[s]
                t0c = 512 * I
                if c == LAST_CHAIN:
                    # final chain: one DVE cast (a single 512-col cast beats
                    # two serial 256-col halves), stores split on both rings
                    nc.vector.tensor_copy(
                        stg[:, t0c : t0c + 512], psums[c][:, 0:512]
                    )
                    nc.sync.dma_start(
                        out_d[s][:, t0c : t0c + 256], stg[:, t0c : t0c + 256]
                    )
                    nc.scalar.dma_start(
                        out_d[s][:, t0c + 256 : t0c + 512],
                        stg[:, t0c + 256 : t0c + 512],
                    )
                    continue
                nc.vector.tensor_copy(
                    stg[:, t0c : t0c + 512], psums[c][:, 0:512]
                )
                if s <= 2:
                    if I == BLOCK_ORDER[3]:  # last block: whole scale ready
                        eng = getattr(nc, STORE_ENG[(s, "all")])
                        eng.dma_start(out_d[s], stg[:])
                    continue
                if I == BLOCK_ORDER[0]:
                    continue  # held in stg; stored merged with next block
                if I == BLOCK_ORDER[1]:
                    # merged store of the two first-processed blocks
                    a = 512 * min(I, BLOCK_ORDER[0])
                    b = 512 * max(I, BLOCK_ORDER[0]) + 512
                    eng = getattr(nc, STORE_ENG[(s, "m32")])
                    eng.dma_start(out_d[s][:, a:b], stg[:, a:b])
                else:
                    eng = getattr(nc, STORE_ENG[(s, I)])
                    eng.dma_start(
                        out_d[s][:, t0c : t0c + 512], stg[:, t0c : t0c + 512]
                    )

    nc.compile()
    return nc


def _program():
    global _NC_CACHE
    if _NC_CACHE is None:
        _NC_CACHE = _build_program()
    return _NC_CACHE


# ----------------------------------------------------------------- entry
def kernel(x: np.ndarray) -> np.ndarray:
    """x: [16, 2048, 64] float32 -> [16, 2048, 64, 5] float32"""
    global LAST_EXEC_NS
    import ml_dtypes
    from concourse.bass_utils import run_bass_kernel_spmd

    x = np.asarray(x)
    n, t, c = x.shape
    assert (t, n * c) == (T, B), (x.shape,)

    X = x.transpose(1, 0, 2).reshape(T, B).astype(np.float32)
    _, wbuf = _consts()
    in_maps = []
    for core in range(N_CORES):
        xc = X[:, core * BPC : (core + 1) * BPC]  # [2048, 128]
        xc = (
            xc.reshape(NQ, 128, BPC)
            .transpose(1, 0, 2)
            .reshape(128, NQ * BPC)
            .astype(ml_dtypes.bfloat16)
        )
        in_maps.append({"x": np.ascontiguousarray(xc), "w": wbuf})

    nc = _program()
    trace = bool(int(os.environ.get("CWT_TRACE", "0")))
    res = run_bass_kernel_spmd(nc, in_maps, list(range(N_CORES)), trace=trace)
    if trace:
        LAST_EXEC_NS = res.exec_time_ns
        globals()["LAST_RESULTS"] = res

    # per-core out: [5, 128, 2048] bf16 (b-local, t) -> Y [5, T, B] fp32
    Y = np.empty((NSC, T, B), np.float32)
    for core in range(N_CORES):
        o = np.asarray(res.results[core]["out"]).astype(np.float32)
        Y[:, :, core * BPC : (core + 1) * BPC] = o.transpose(0, 2, 1)
    return np.ascontiguousarray(
        Y.reshape(NSC, T, n, c).transpose(2, 1, 3, 0).astype(np.float32)
    )
